# revision 91
# baseline (speedup 1.0000x reference)
"""Bahdanau attention (B=64, T=2048, D=U=512) on 8 trn2 NeuronCores.

Data-parallel over batch: 8 batches per core. Per core, per batch:
  encT[d,t]  <- xbar DMA-transpose of the bf16 encoder slice straight from
                DRAM (encoder is pre-cast to bf16 on the host)
  k^T[u,t]   <- PE matmul, lhsT=W2 chunks, rhs=encT (bf16 in, fp32 accum)
  score^T    <- ScalarE tanh(k^T + q_bias[u]), q fused via per-partition bias
  logits     <- PE matmuls, lhsT=V (M=1): the four t-tiles run CONCURRENTLY
                in separate 32-column PE strips via tile_position col-tiling
                (~4x less PE time than sequential M=1 matmuls)
  exp        <- ScalarE exp, unnormalized (|logits| <= ||V||_1 + |bV| ~ 18,
                so exp stays comfortably inside fp32/bf16 range and the
                softmax max-subtraction can be dropped)
  context    <- VectorE scalar_tensor_tensor(encT * w_bcast) with fused
                row-sum accumulation, weights broadcast across partitions
                via a DRAM bounce. The LAST batch instead replicates weights
                with row-tiled PE ones-matmuls (PE+PSUM idle at the tail),
                cutting the bounce+broadcast latency out of the kernel tail.
Normalization (1/sum) is deferred: one reciprocal instruction serves all 8
batches; attention weights and the transposed context are scaled at the tail.
q = dh @ W1 + (b1 + b2) is computed once per core on PE. bV is ignored:
softmax is invariant to constant logit shifts.
"""

import numpy as np
import ml_dtypes

import concourse.bacc as bacc
import concourse.bass as bass
import concourse.mybir as mybir
import concourse.tile as tile
from concourse.bass_utils import run_bass_kernel_spmd
from concourse.masks import make_identity

B, T, D, U = 64, 2048, 512, 512
N_CORES = 8
NB = B // N_CORES          # batches per core
DC = D // 128              # d chunks
UC = U // 128              # u chunks
TT = T // 512              # t tiles
F32 = mybir.dt.float32
BF16 = mybir.dt.bfloat16
BF16_NP = ml_dtypes.bfloat16


def build_nc(skip=()):
    skip = set(skip)
    nc = bacc.Bacc(None, target_bir_lowering=False)

    enc = nc.dram_tensor("enc", [NB, T, D], BF16, kind="ExternalInput")
    dhbt = nc.dram_tensor("dhbt", [128, DC, NB + 1], F32, kind="ExternalInput")
    w1 = nc.dram_tensor("w1", [128, DC, U], BF16, kind="ExternalInput")
    w2 = nc.dram_tensor("w2", [128, DC, U], BF16, kind="ExternalInput")
    v = nc.dram_tensor("v", [128, UC], BF16, kind="ExternalInput")
    ctx_out = nc.dram_tensor("ctx_out", [NB, D], F32, kind="ExternalOutput")
    attn_out = nc.dram_tensor("attn_out", [NB, T], F32, kind="ExternalOutput")

    Tanh = mybir.ActivationFunctionType.Tanh
    Exp = mybir.ActivationFunctionType.Exp
    Add = mybir.AluOpType.add
    Mult = mybir.AluOpType.mult
    X = mybir.AxisListType.X

    with tile.TileContext(nc) as tc:
        with (
            tc.tile_pool(name="consts", bufs=1) as consts,
            tc.tile_pool(name="enc_pool", bufs=6) as enc_pool,
            tc.tile_pool(name="score_pool", bufs=2) as score_pool,
            tc.tile_pool(name="wb_pool", bufs=3) as wb_pool,
            tc.tile_pool(name="small", bufs=2) as small,
            tc.tile_pool(name="drp", bufs=2, space="DRAM") as drp,
            tc.tile_pool(name="kps_pool", bufs=3, space="PSUM") as kps_pool,
            tc.tile_pool(name="lps_pool", bufs=1, space="PSUM") as lps_pool,
        ):
            # ---- constants ----
            w1_sb = consts.tile([128, DC, U], BF16)
            w2_sb = consts.tile([128, DC, U], BF16)
            v_sb = consts.tile([128, UC], BF16)
            ident = consts.tile([128, 128], F32)
            nc.sync.dma_start(out=w2_sb, in_=w2[:, :, :])
            nc.sync.dma_start(out=w1_sb, in_=w1[:, :, :])
            nc.sync.dma_start(out=v_sb, in_=v[:, :])
            make_identity(nc, ident)
            ones128 = consts.tile([128, 128], BF16)
            nc.vector.memset(ones128, 1.0)

            # ---- q projection: q_bias[u, b] = dh @ W1 + (b1 + b2) ----
            # dhbt arrives pre-transposed from the host: [d%128, dc, j] with
            # j in 0..NB-1 the batch (decoder_hidden) and j == NB the b1+b2
            # row, so no on-device transpose sits on the startup path.
            dhbT = consts.tile([128, DC, NB + 1], F32)
            dhT16 = consts.tile([128, DC, NB], BF16)
            qb = consts.tile([128, UC, NB], F32)
            nc.sync.dma_start(out=dhbT, in_=dhbt[:, :, :])
            nc.vector.tensor_copy(dhT16, dhbT[:, :, :NB])
            for uc in range(UC):
                qps = lps_pool.tile([128, NB], F32, tag="lps")
                for dc in range(DC):
                    nc.tensor.matmul(
                        qps,
                        w1_sb[:, dc, uc * 128 : (uc + 1) * 128],
                        dhT16[:, dc, :],
                        start=(dc == 0),
                        stop=(dc == DC - 1),
                    )
                # add (b1+b2) chunk: dhbT[:, uc, NB] holds (b1+b2)[uc*128+p]
                nc.vector.tensor_scalar_add(
                    qb[:, uc, :], qps, dhbT[:, uc, NB : NB + 1]
                )

            # ---- context accumulator [d%128, b*DC + dc] ----
            ctx_all = consts.tile([128, NB * DC], F32)
            # unnormalized exp weights, bounced through DRAM per batch
            wall = drp.tile([NB, T], BF16, bufs=1)
            rdram = drp.tile([NB, 1], F32, bufs=1)

            for b in range(NB):
                # transposed encoder load: encT[d%128, dc, t]
                encT = enc_pool.tile([128, DC, T], BF16, tag="encT")
                for dc in range(DC):
                    nc.sync.dma_start_transpose(
                        out=encT[:, dc, :],
                        in_=enc[b, :, dc * 128 : (dc + 1) * 128],
                    )

                # k^T = (enc @ W2)^T ; score^T = tanh(k^T + q_bias)
                # kps holds two 512-wide t-tiles (2 PSUM banks) so one tanh
                # covers 1024 elements, halving ACT op count.
                scoreT = score_pool.tile([128, UC, T], BF16, tag="scoreT")
                for th in range(TT // 2):
                    for uc in range(UC):
                        kps = kps_pool.tile([128, 2, 512], F32, tag="kps")
                        if "kproj" not in skip:
                            for half in range(2):
                                tt = th * 2 + half
                                for dc in range(DC):
                                    nc.tensor.matmul(
                                        kps[:, half, :],
                                        w2_sb[:, dc, uc * 128 : (uc + 1) * 128],
                                        encT[:, dc, tt * 512 : (tt + 1) * 512],
                                        start=(dc == 0),
                                        stop=(dc == DC - 1),
                                    )
                        if "tanh" not in skip:
                            nc.scalar.activation(
                                scoreT[:, uc, th * 1024 : (th + 1) * 1024],
                                kps.rearrange("p a b -> p (a b)"),
                                Tanh,
                                bias=qb[:, uc, b : b + 1],
                            )
                        else:
                            nc.vector.memset(
                                scoreT[:, uc, th * 1024 : (th + 1) * 1024], 0.01
                            )

                # logits = score^T . V -> unnormalized exp weights (bf16);
                # normalization deferred to the tail pass. The four t-tiles'
                # M=1 matmuls run CONCURRENTLY in separate 32-column PE
                # strips (tile_position col-tiling), cutting logits PE time
                # ~4x; their [1, 512] rows land at partitions 0/32/64/96 of
                # one PSUM bank and exp handles all four in one strided op.
                lps = lps_pool.tile([128, 512], F32, tag="lps")
                if "logits" not in skip:
                    for uc in range(UC):
                        for tt in range(TT):
                            nc.tensor.matmul(
                                lps[32 * tt : 32 * tt + 1, :],
                                v_sb[:, uc : uc + 1],
                                scoreT[:, uc, tt * 512 : (tt + 1) * 512],
                                start=(uc == 0),
                                stop=(uc == UC - 1),
                                tile_position=(0, 32 * tt),
                            )
                else:
                    nc.vector.memset(lps, 0.01)
                exp4 = small.tile([128, 512], BF16, tag="exp4")
                for tt in range(TT):
                    nc.scalar.activation(
                        exp4[32 * tt : 32 * tt + 1, :],
                        lps[32 * tt : 32 * tt + 1, :],
                        Exp,
                    )

                # bounce to DRAM (feeds the attention tail reload), then
                # broadcast across partitions and run the context reduce
                nc.gpsimd.dma_start(
                    out=wall[b : b + 1, :].rearrange(
                        "o (a e) -> (o a) e", e=512
                    ),
                    in_=exp4.rearrange("(a c) b -> a c b", c=32)[:, 0, :],
                )
                if "context" not in skip and b < NB - 1:
                    w_b = wb_pool.tile([128, T], BF16, tag="w_b")
                    nc.gpsimd.dma_start(
                        out=w_b, in_=wall[b : b + 1, :].broadcast_to([128, T])
                    )
                    prod = wb_pool.tile([128, T], BF16, tag="prod")
                    for dc in range(DC):
                        nc.vector.scalar_tensor_tensor(
                            out=prod,
                            in0=encT[:, dc, :],
                            scalar=1.0,
                            in1=w_b,
                            op0=Mult,
                            op1=Mult,
                            accum_out=ctx_all[:, b * DC + dc : b * DC + dc + 1],
                        )
                if "context" not in skip and b == NB - 1:
                    # Last batch: its context chain IS the kernel tail, so
                    # skip the DRAM bounce+broadcast latency by replicating
                    # the weights with a PE ones-matmul straight from exp4
                    # (PE and PSUM are idle here). t-halves accumulate to
                    # separate columns, folded with one add.
                    ctx_b7 = consts.tile([128, DC], F32)
                    prod7 = wb_pool.tile([128, T], BF16, tag="prod")
                    for half in range(2):
                        wps = kps_pool.tile([128, 2, 512], F32, tag="kps")
                        for j in range(2):
                            tq = half * 2 + j
                            nc.tensor.matmul(
                                wps[:, j, :],
                                ones128[32 * tq : 32 * tq + 1, :],
                                exp4[32 * tq : 32 * tq + 1, :],
                                start=True,
                                stop=True,
                                tile_position=(32 * tq, 0),
                            )
                        wflat = wps.rearrange("p a b -> p (a b)")
                        hs = slice(half * 1024, (half + 1) * 1024)
                        for dc in range(DC):
                            acc = (
                                ctx_all[:, b * DC + dc : b * DC + dc + 1]
                                if half == 0
                                else ctx_b7[:, dc : dc + 1]
                            )
                            nc.vector.scalar_tensor_tensor(
                                out=prod7[:, hs],
                                in0=encT[:, dc, hs],
                                scalar=1.0,
                                in1=wflat,
                                op0=Mult,
                                op1=Mult,
                                accum_out=acc,
                            )
                    nc.vector.tensor_add(
                        ctx_all[:, b * DC : (b + 1) * DC],
                        ctx_all[:, b * DC : (b + 1) * DC],
                        ctx_b7,
                    )

                # Attention-weight normalization. Row sums land in one
                # [hb, 2] tile so a single reciprocal instruction serves all
                # batches; the normalize runs in bf16 (DVE 2x mode) with a
                # tail cast-DMA to f32.
                hb = NB // 2
                if b == hb - 1:
                    ssum2 = consts.tile([hb, 2], F32)
                    rinv2 = consts.tile([hb, 2], F32)
                    exph0 = consts.tile([hb, T], BF16)
                    nc.sync.dma_start(out=exph0, in_=wall[:hb, :])
                    nc.vector.tensor_reduce(
                        ssum2[:, 0:1], exph0, axis=X, op=Add
                    )
                if b == NB - 1:
                    # End-of-kernel ops run on ScalarE (idle after the last
                    # tanh/exp; nothing behind them in its queue to stall)
                    # so VectorE's queue ends with the reciprocal.
                    exph1 = consts.tile([hb, T], BF16)
                    nc.sync.dma_start(out=exph1, in_=wall[hb:, :])
                    scr1 = consts.tile([hb, T], BF16)
                    nc.scalar.activation(
                        scr1,
                        exph1,
                        mybir.ActivationFunctionType.Identity,
                        accum_out=ssum2[:, 1:2],
                    )
                    nc.vector.reciprocal(rinv2, ssum2)
                    attnh0 = consts.tile([hb, T], F32)
                    attnh1 = consts.tile([hb, T], F32)
                    nc.scalar.activation(
                        attnh0,
                        exph0,
                        mybir.ActivationFunctionType.Copy,
                        scale=rinv2[:, 0:1],
                    )
                    nc.scalar.activation(
                        attnh1,
                        exph1,
                        mybir.ActivationFunctionType.Copy,
                        scale=rinv2[:, 1:2],
                    )
                    nc.gpsimd.dma_start(out=attn_out[:hb, :], in_=attnh0)
                    nc.gpsimd.dma_start(out=attn_out[hb:, :], in_=attnh1)
                    # rdram[b] = rinv2[b % hb, b // hb]
                    nc.sync.dma_start(
                        out=rdram.rearrange("(h i) o -> i (h o)", i=hb),
                        in_=rinv2[:, :],
                    )

            # rinv replicated x4 so each (b, dc) row of the transposed
            # context picks up its batch's 1/sum. One strided-partition load
            # per dc (zero-step DRAM reads silently collapse, so no 0-step
            # replication here).
            rinv32 = consts.tile([NB * DC, 1], F32)
            for dc in range(DC):
                nc.sync.dma_start(
                    out=rinv32.rearrange("(a c) o -> c a o", c=DC)[dc],
                    in_=rdram[:, :],
                )

            # ---- context transpose [128, NB*DC] -> [NB*DC, 128], scale, store
            cps = lps_pool.tile([NB * DC, 128], F32, tag="lps")
            nc.tensor.transpose(cps, ctx_all, ident)
            ctxT = consts.tile([NB * DC, 128], F32)
            nc.scalar.activation(
                ctxT,
                cps,
                mybir.ActivationFunctionType.Copy,
                scale=rinv32,
            )
            nc.sync.dma_start(
                out=ctx_out.rearrange("b (c e) -> (b c) e", c=DC), in_=ctxT
            )

    nc.finalize()
    return nc


_NC_CACHE = None


def _get_nc():
    global _NC_CACHE
    if _NC_CACHE is None:
        _NC_CACHE = build_nc()
    return _NC_CACHE


RUN_KWARGS = {}
LAST_RESULT = None


def kernel(decoder_hidden, encoder_output, W1, b1, W2, b2, V, bV):
    global LAST_RESULT
    decoder_hidden = np.asarray(decoder_hidden, np.float32)
    encoder_output = np.asarray(encoder_output, np.float32)
    W1 = np.asarray(W1, np.float32)
    b1 = np.asarray(b1, np.float32)
    W2 = np.asarray(W2, np.float32)
    b2 = np.asarray(b2, np.float32)
    V = np.asarray(V, np.float32)

    enc16 = encoder_output.astype(BF16_NP)
    b12 = (b1 + b2).astype(np.float32)
    w1_h = np.ascontiguousarray(
        W1.reshape(DC, 128, U).transpose(1, 0, 2).astype(BF16_NP)
    )
    w2_h = np.ascontiguousarray(
        W2.reshape(DC, 128, U).transpose(1, 0, 2).astype(BF16_NP)
    )
    v_h = np.ascontiguousarray(V[:, 0].reshape(UC, 128).T.astype(BF16_NP))

    in_maps = []
    for i in range(N_CORES):
        sl = slice(i * NB, (i + 1) * NB)
        dhb_i = np.concatenate([decoder_hidden[sl], b12[None, :]], axis=0)
        # pre-transposed for the device: [d%128, dc, row]
        dhbt_i = np.ascontiguousarray(
            dhb_i.T.reshape(DC, 128, NB + 1).transpose(1, 0, 2)
        )
        in_maps.append(
            {
                "enc": np.ascontiguousarray(enc16[sl]),
                "dhbt": dhbt_i,
                "w1": w1_h,
                "w2": w2_h,
                "v": v_h,
            }
        )

    nc = _get_nc()
    r = run_bass_kernel_spmd(
        nc, in_maps, core_ids=list(range(N_CORES)), **RUN_KWARGS
    )
    LAST_RESULT = r
    context = np.concatenate([r.results[i]["ctx_out"] for i in range(N_CORES)], 0)
    attn = np.concatenate([r.results[i]["attn_out"] for i in range(N_CORES)], 0)
    return context, attn.reshape(B, T, 1)


# revision 96
# speedup vs baseline: 1.0008x; 1.0008x over previous
"""Bahdanau attention (B=64, T=2048, D=U=512) on 8 trn2 NeuronCores.

Data-parallel over batch: 8 batches per core. Per core, per batch:
  encT[d,t]  <- xbar DMA-transpose of the bf16 encoder slice straight from
                DRAM (encoder is pre-cast to bf16 on the host)
  k^T[u,t]   <- PE matmul, lhsT=W2 chunks, rhs=encT (bf16 in, fp32 accum)
  score^T    <- ScalarE tanh(k^T + q_bias[u]), q fused via per-partition bias
  logits     <- PE matmuls, lhsT=V (M=1): the four t-tiles run CONCURRENTLY
                in separate 32-column PE strips via tile_position col-tiling
                (~4x less PE time than sequential M=1 matmuls)
  exp        <- ScalarE exp, unnormalized (|logits| <= ||V||_1 + |bV| ~ 18,
                so exp stays comfortably inside fp32/bf16 range and the
                softmax max-subtraction can be dropped)
  context    <- VectorE scalar_tensor_tensor(encT * w_bcast) with fused
                row-sum accumulation, weights broadcast across partitions
                via a DRAM bounce. The LAST batch instead replicates weights
                with row-tiled PE ones-matmuls (PE+PSUM idle at the tail),
                cutting the bounce+broadcast latency out of the kernel tail.
Normalization (1/sum) is deferred: one reciprocal instruction serves all 8
batches; attention weights and the transposed context are scaled at the tail.
q = dh @ W1 + (b1 + b2) is computed once per core on PE. bV is ignored:
softmax is invariant to constant logit shifts.
"""

import numpy as np
import ml_dtypes

import concourse.bacc as bacc
import concourse.bass as bass
import concourse.mybir as mybir
import concourse.tile as tile
from concourse.bass_utils import run_bass_kernel_spmd
from concourse.masks import make_identity

B, T, D, U = 64, 2048, 512, 512
N_CORES = 8
NB = B // N_CORES          # batches per core
DC = D // 128              # d chunks
UC = U // 128              # u chunks
TT = T // 512              # t tiles
F32 = mybir.dt.float32
BF16 = mybir.dt.bfloat16
BF16_NP = ml_dtypes.bfloat16


def build_nc(skip=()):
    skip = set(skip)
    nc = bacc.Bacc(None, target_bir_lowering=False)

    enc = nc.dram_tensor("enc", [NB, T, D], BF16, kind="ExternalInput")
    dhbt = nc.dram_tensor("dhbt", [128, DC, NB + 1], F32, kind="ExternalInput")
    w1 = nc.dram_tensor("w1", [128, DC, U], BF16, kind="ExternalInput")
    w2 = nc.dram_tensor("w2", [128, DC, U], BF16, kind="ExternalInput")
    v = nc.dram_tensor("v", [128, UC], BF16, kind="ExternalInput")
    ctx_out = nc.dram_tensor("ctx_out", [NB, D], F32, kind="ExternalOutput")
    attn_out = nc.dram_tensor("attn_out", [NB, T], F32, kind="ExternalOutput")

    Tanh = mybir.ActivationFunctionType.Tanh
    Exp = mybir.ActivationFunctionType.Exp
    Add = mybir.AluOpType.add
    Mult = mybir.AluOpType.mult
    X = mybir.AxisListType.X

    with tile.TileContext(nc) as tc:
        with (
            tc.tile_pool(name="consts", bufs=1) as consts,
            tc.tile_pool(name="enc_pool", bufs=6) as enc_pool,
            tc.tile_pool(name="score_pool", bufs=2) as score_pool,
            tc.tile_pool(name="wb_pool", bufs=3) as wb_pool,
            tc.tile_pool(name="small", bufs=2) as small,
            tc.tile_pool(name="drp", bufs=2, space="DRAM") as drp,
            tc.tile_pool(name="kps_pool", bufs=3, space="PSUM") as kps_pool,
            tc.tile_pool(name="lps_pool", bufs=1, space="PSUM") as lps_pool,
        ):
            # ---- constants ----
            w1_sb = consts.tile([128, DC, U], BF16)
            w2_sb = consts.tile([128, DC, U], BF16)
            v_sb = consts.tile([128, UC], BF16)
            ident = consts.tile([128, 128], F32)
            nc.sync.dma_start(out=w2_sb, in_=w2[:, :, :])
            nc.sync.dma_start(out=w1_sb, in_=w1[:, :, :])
            nc.sync.dma_start(out=v_sb, in_=v[:, :])
            make_identity(nc, ident)
            ones128 = consts.tile([128, 128], BF16)
            nc.vector.memset(ones128, 1.0)

            # ---- q projection: q_bias[u, b] = dh @ W1 + (b1 + b2) ----
            # dhbt arrives pre-transposed from the host: [d%128, dc, j] with
            # j in 0..NB-1 the batch (decoder_hidden) and j == NB the b1+b2
            # row, so no on-device transpose sits on the startup path.
            dhbT = consts.tile([128, DC, NB + 1], F32)
            dhT16 = consts.tile([128, DC, NB], BF16)
            qb = consts.tile([128, UC, NB], F32)
            nc.sync.dma_start(out=dhbT, in_=dhbt[:, :, :])
            nc.vector.tensor_copy(dhT16, dhbT[:, :, :NB])
            for uc in range(UC):
                qps = lps_pool.tile([128, NB], F32, tag="lps")
                for dc in range(DC):
                    nc.tensor.matmul(
                        qps,
                        w1_sb[:, dc, uc * 128 : (uc + 1) * 128],
                        dhT16[:, dc, :],
                        start=(dc == 0),
                        stop=(dc == DC - 1),
                    )
                # add (b1+b2) chunk: dhbT[:, uc, NB] holds (b1+b2)[uc*128+p]
                nc.vector.tensor_scalar_add(
                    qb[:, uc, :], qps, dhbT[:, uc, NB : NB + 1]
                )

            # ---- context accumulator [d%128, b*DC + dc] ----
            ctx_all = consts.tile([128, NB * DC], F32)
            # unnormalized exp weights, bounced through DRAM per batch
            wall = drp.tile([NB, T], BF16, bufs=1)

            for b in range(NB):
                # transposed encoder load: encT[d%128, dc, t]
                encT = enc_pool.tile([128, DC, T], BF16, tag="encT")
                for dc in range(DC):
                    nc.sync.dma_start_transpose(
                        out=encT[:, dc, :],
                        in_=enc[b, :, dc * 128 : (dc + 1) * 128],
                    )

                # k^T = (enc @ W2)^T ; score^T = tanh(k^T + q_bias)
                # kps holds two 512-wide t-tiles (2 PSUM banks) so one tanh
                # covers 1024 elements, halving ACT op count.
                scoreT = score_pool.tile([128, UC, T], BF16, tag="scoreT")
                for th in range(TT // 2):
                    for uc in range(UC):
                        kps = kps_pool.tile([128, 2, 512], F32, tag="kps")
                        if "kproj" not in skip:
                            for half in range(2):
                                tt = th * 2 + half
                                for dc in range(DC):
                                    nc.tensor.matmul(
                                        kps[:, half, :],
                                        w2_sb[:, dc, uc * 128 : (uc + 1) * 128],
                                        encT[:, dc, tt * 512 : (tt + 1) * 512],
                                        start=(dc == 0),
                                        stop=(dc == DC - 1),
                                    )
                        if "tanh" not in skip:
                            nc.scalar.activation(
                                scoreT[:, uc, th * 1024 : (th + 1) * 1024],
                                kps.rearrange("p a b -> p (a b)"),
                                Tanh,
                                bias=qb[:, uc, b : b + 1],
                            )
                        else:
                            nc.vector.memset(
                                scoreT[:, uc, th * 1024 : (th + 1) * 1024], 0.01
                            )

                # logits = score^T . V -> unnormalized exp weights (bf16);
                # normalization deferred to the tail pass. The four t-tiles'
                # M=1 matmuls run CONCURRENTLY in separate 32-column PE
                # strips (tile_position col-tiling), cutting logits PE time
                # ~4x; their [1, 512] rows land at partitions 0/32/64/96 of
                # one PSUM bank and exp handles all four in one strided op.
                lps = lps_pool.tile([128, 512], F32, tag="lps")
                if "logits" not in skip:
                    for uc in range(UC):
                        for tt in range(TT):
                            nc.tensor.matmul(
                                lps[32 * tt : 32 * tt + 1, :],
                                v_sb[:, uc : uc + 1],
                                scoreT[:, uc, tt * 512 : (tt + 1) * 512],
                                start=(uc == 0),
                                stop=(uc == UC - 1),
                                tile_position=(0, 32 * tt),
                            )
                else:
                    nc.vector.memset(lps, 0.01)
                exp4 = small.tile([128, 512], BF16, tag="exp4")
                for tt in range(TT):
                    nc.scalar.activation(
                        exp4[32 * tt : 32 * tt + 1, :],
                        lps[32 * tt : 32 * tt + 1, :],
                        Exp,
                    )

                # bounce to DRAM (feeds the attention tail reload), then
                # broadcast across partitions and run the context reduce
                nc.gpsimd.dma_start(
                    out=wall[b : b + 1, :].rearrange(
                        "o (a e) -> (o a) e", e=512
                    ),
                    in_=exp4.rearrange("(a c) b -> a c b", c=32)[:, 0, :],
                )
                if "context" not in skip and b < NB - 1:
                    w_b = wb_pool.tile([128, T], BF16, tag="w_b")
                    nc.gpsimd.dma_start(
                        out=w_b, in_=wall[b : b + 1, :].broadcast_to([128, T])
                    )
                    prod = wb_pool.tile([128, T], BF16, tag="prod")
                    for dc in range(DC):
                        nc.vector.scalar_tensor_tensor(
                            out=prod,
                            in0=encT[:, dc, :],
                            scalar=1.0,
                            in1=w_b,
                            op0=Mult,
                            op1=Mult,
                            accum_out=ctx_all[:, b * DC + dc : b * DC + dc + 1],
                        )
                if "context" not in skip and b == NB - 1:
                    # Last batch: its context chain IS the kernel tail, so
                    # skip the DRAM bounce+broadcast latency by replicating
                    # the weights with a PE ones-matmul straight from exp4
                    # (PE and PSUM are idle here). t-halves accumulate to
                    # separate columns, folded with one add.
                    ctx_b7 = consts.tile([128, DC], F32)
                    prod7 = wb_pool.tile([128, T], BF16, tag="prod")
                    for half in range(2):
                        wps = kps_pool.tile([128, 2, 512], F32, tag="kps")
                        for j in range(2):
                            tq = half * 2 + j
                            nc.tensor.matmul(
                                wps[:, j, :],
                                ones128[32 * tq : 32 * tq + 1, :],
                                exp4[32 * tq : 32 * tq + 1, :],
                                start=True,
                                stop=True,
                                tile_position=(32 * tq, 0),
                            )
                        wflat = wps.rearrange("p a b -> p (a b)")
                        hs = slice(half * 1024, (half + 1) * 1024)
                        for dc in range(DC):
                            acc = (
                                ctx_all[:, b * DC + dc : b * DC + dc + 1]
                                if half == 0
                                else ctx_b7[:, dc : dc + 1]
                            )
                            nc.vector.scalar_tensor_tensor(
                                out=prod7[:, hs],
                                in0=encT[:, dc, hs],
                                scalar=1.0,
                                in1=wflat,
                                op0=Mult,
                                op1=Mult,
                                accum_out=acc,
                            )
                    nc.vector.tensor_add(
                        ctx_all[:, b * DC : (b + 1) * DC],
                        ctx_all[:, b * DC : (b + 1) * DC],
                        ctx_b7,
                    )

                # Attention-weight normalization on 8 partitions: one
                # reload + free-dim reduce per half, then a single
                # reciprocal, a single ScalarE scale over all 8 batches and
                # one output DMA. Tail ops sit at the end of each engine's
                # queue, so nothing can stall behind them.
                hb = NB // 2
                if b == hb - 1:
                    exph = consts.tile([NB, T], BF16)
                    ssum8 = consts.tile([NB, 1], F32)
                    rinv8 = consts.tile([NB, 1], F32)
                    nc.sync.dma_start(out=exph[:hb], in_=wall[:hb, :])
                if b == NB - 1:
                    nc.sync.dma_start(out=exph[hb:], in_=wall[hb:, :])
                    # one reduce over all 8 batches (DVE needs 32-aligned
                    # partition bases, so halves can't reduce separately)
                    nc.vector.tensor_reduce(ssum8, exph, axis=X, op=Add)
                    nc.vector.reciprocal(rinv8, ssum8)
                    attnh = consts.tile([NB, T], F32)
                    nc.scalar.activation(
                        attnh,
                        exph,
                        mybir.ActivationFunctionType.Copy,
                        scale=rinv8,
                    )
                    nc.sync.dma_start(out=attn_out[:, :], in_=attnh)

            # rinv replicated x4 so each (b, dc) row of the transposed
            # context picks up its batch's 1/sum: strided-partition
            # SBUF->SBUF remap loads (no DRAM round-trip; zero-step reads
            # would silently collapse, hence one load per dc).
            rinv32 = consts.tile([NB * DC, 1], F32)
            for dc in range(DC):
                nc.sync.dma_start(
                    out=rinv32.rearrange("(a c) o -> c a o", c=DC)[dc],
                    in_=rinv8[:, :],
                )

            # ---- context transpose [128, NB*DC] -> [NB*DC, 128], scale, store
            cps = lps_pool.tile([NB * DC, 128], F32, tag="lps")
            nc.tensor.transpose(cps, ctx_all, ident)
            ctxT = consts.tile([NB * DC, 128], F32)
            nc.scalar.activation(
                ctxT,
                cps,
                mybir.ActivationFunctionType.Copy,
                scale=rinv32,
            )
            nc.sync.dma_start(
                out=ctx_out.rearrange("b (c e) -> (b c) e", c=DC), in_=ctxT
            )

    nc.finalize()
    return nc


_NC_CACHE = None


def _get_nc():
    global _NC_CACHE
    if _NC_CACHE is None:
        _NC_CACHE = build_nc()
    return _NC_CACHE


RUN_KWARGS = {}
LAST_RESULT = None


def kernel(decoder_hidden, encoder_output, W1, b1, W2, b2, V, bV):
    global LAST_RESULT
    decoder_hidden = np.asarray(decoder_hidden, np.float32)
    encoder_output = np.asarray(encoder_output, np.float32)
    W1 = np.asarray(W1, np.float32)
    b1 = np.asarray(b1, np.float32)
    W2 = np.asarray(W2, np.float32)
    b2 = np.asarray(b2, np.float32)
    V = np.asarray(V, np.float32)

    enc16 = encoder_output.astype(BF16_NP)
    b12 = (b1 + b2).astype(np.float32)
    w1_h = np.ascontiguousarray(
        W1.reshape(DC, 128, U).transpose(1, 0, 2).astype(BF16_NP)
    )
    w2_h = np.ascontiguousarray(
        W2.reshape(DC, 128, U).transpose(1, 0, 2).astype(BF16_NP)
    )
    v_h = np.ascontiguousarray(V[:, 0].reshape(UC, 128).T.astype(BF16_NP))

    in_maps = []
    for i in range(N_CORES):
        sl = slice(i * NB, (i + 1) * NB)
        dhb_i = np.concatenate([decoder_hidden[sl], b12[None, :]], axis=0)
        # pre-transposed for the device: [d%128, dc, row]
        dhbt_i = np.ascontiguousarray(
            dhb_i.T.reshape(DC, 128, NB + 1).transpose(1, 0, 2)
        )
        in_maps.append(
            {
                "enc": np.ascontiguousarray(enc16[sl]),
                "dhbt": dhbt_i,
                "w1": w1_h,
                "w2": w2_h,
                "v": v_h,
            }
        )

    nc = _get_nc()
    r = run_bass_kernel_spmd(
        nc, in_maps, core_ids=list(range(N_CORES)), **RUN_KWARGS
    )
    LAST_RESULT = r
    context = np.concatenate([r.results[i]["ctx_out"] for i in range(N_CORES)], 0)
    attn = np.concatenate([r.results[i]["attn_out"] for i in range(N_CORES)], 0)
    return context, attn.reshape(B, T, 1)
